# revision 47
# baseline (speedup 1.0000x reference)
"""Trainium2 Bass kernel for nn_CGNNExpert (GATv2-based GNN, 3 layers).

Sharding: nodes block-partitioned across 8 cores by destination; each core
owns its nodes' incoming edges. Per-node slot lists (self-loop + in-edges,
padded to a multiple of 4) follow a degree-rank template shared by all cores
so a single SPMD program serves every core. Channel-major on-chip layout:
per layer: local xl/xr matmuls -> AllGather of xl token rows -> SBUF-source
dma_gather of xl[src] (token-interleaved stripes) -> u = xl_g + xr + ew*We
on DVE -> LeakyReLU -> per-head logits via block-diagonal att matmuls
(pre-expanded across partitions) -> exp (sentinel tokens force pad-slot
logits <= -120 so exp == 0) -> AP-grouped segment reduces for softmax denom
and message sums.
"""

import numpy as np

N, E, IN, HID, HEADS, L = 10000, 320000, 128, 256, 16, 3
C = HID // HEADS
NCORES = 8
NLOC = N // NCORES  # 1250
NLOC_X = NLOC + 2   # local xl slab rows (+2 sentinel rows fed into the AG)
NTOK = NCORES * NLOC_X  # 10016 gatherable token rows
CHUNK_TARGET = 512   # one dma_gather per chunk (>512 idxs breaks on HW)
SUPER = 4            # chunks per super-chunk (shared big tiles / DVE passes)
NTILE = (NLOC + 127) // 128  # 10 node tiles per core
NLOC_PAD = NTILE * 128       # 1280
P = 128
NSTRIPE = (NTOK + P - 1) // P  # 79 table rank-stripes (78 full + 32-row tail)


def _roundup(x, m):
    return (x + m - 1) // m * m


# ----------------------------------------------------------------------------
# host-side schedule
# ----------------------------------------------------------------------------

def build_schedule(edge_index, edge_weight):
    src = np.asarray(edge_index[0])
    dst = np.asarray(edge_index[1])
    ew = np.asarray(edge_weight).reshape(-1).astype(np.float32)
    owner = dst // NLOC
    dloc = dst % NLOC

    deg = np.zeros((NCORES, NLOC), np.int64)
    np.add.at(deg, (owner, dloc), 1)

    # shared structural template: elementwise max of per-core desc-sorted
    # degree profiles, +1 self loop, rounded up to a multiple of 4
    prof = np.sort(deg, axis=1)[:, ::-1]
    tmpl_pads = ((prof.max(axis=0) + 1 + 3) // 4 * 4).astype(np.int64)

    # chunks: node-aligned slot ranges, each exactly CHUNK_TARGET slots
    chunks = []
    i = 0
    while i < NLOC:
        s = 0
        j = i
        while j < NLOC and s + tmpl_pads[j] <= CHUNK_TARGET:
            s += tmpl_pads[j]
            j += 1
        S_chunk = CHUNK_TARGET
        runs = []
        k = i
        soff = 0
        while k < j:
            k2 = k
            while k2 < j and tmpl_pads[k2] == tmpl_pads[k]:
                k2 += 1
            runs.append(dict(pad=int(tmpl_pads[k]), n=int(k2 - k),
                             slot_off=int(soff), node_off=int(k - i)))
            soff += int(tmpl_pads[k]) * (k2 - k)
            k = k2
        chunks.append(dict(S=int(S_chunk), runs=runs, node0=int(i),
                           n_nodes=int(j - i), tail=int(S_chunk - s)))
        i = j
    S_total = sum(ch["S"] for ch in chunks)

    # per-core permutation: nodes by degree desc (stable)
    perms = np.zeros((NCORES, NLOC), np.int64)
    inv = np.zeros((NCORES, NLOC), np.int64)
    for c in range(NCORES):
        p = np.lexsort((np.arange(NLOC), -deg[c]))
        perms[c] = p
        inv[c, p] = np.arange(NLOC)
    tok_of_global = (np.arange(N) // NLOC) * NLOC_X + inv[
        np.arange(N) // NLOC, np.arange(N) % NLOC
    ]

    # slot offset of each template rank
    slot_off_node = np.zeros(NLOC, np.int64)
    so = 0
    gi = 0
    for ch in chunks:
        for r in ch["runs"]:
            for t in range(r["n"]):
                slot_off_node[gi] = so
                so += r["pad"]
                gi += 1
        so += ch["tail"]
    assert gi == NLOC and so == S_total

    # per-core slot arrays
    order = np.lexsort((src, dloc, owner))
    o_owner, o_dloc, o_src, o_w = owner[order], dloc[order], src[order], ew[order]
    mean_ew = float(ew.mean())  # self-loop fill (host-computed)
    src_tok = np.zeros((NCORES, S_total), np.int32)
    for c in range(NCORES):
        src_tok[c, :] = c * NLOC_X + NLOC  # this core's sentinel row
    ew_slot = np.zeros((NCORES, S_total), np.float32)
    for c in range(NCORES):
        sel = o_owner == c
        e_src = o_src[sel]
        e_w = o_w[sel]
        starts = np.zeros(NLOC + 1, np.int64)
        starts[1:] = np.cumsum(deg[c])
        for rank in range(NLOC):
            node = perms[c][rank]
            so_n = slot_off_node[rank]
            d = int(deg[c][node])
            src_tok[c, so_n] = tok_of_global[c * NLOC + node]  # self loop
            ew_slot[c, so_n] = mean_ew
            e0 = starts[node]
            src_tok[c, so_n + 1:so_n + 1 + d] = tok_of_global[e_src[e0:e0 + d]]
            ew_slot[c, so_n + 1:so_n + 1 + d] = e_w[e0:e0 + d]
    return dict(chunks=chunks, S_total=int(S_total), perms=perms,
                src_tok=src_tok, ew_slot=ew_slot, deg=deg)


def wrap_idx16(idx):
    """[S] -> [128, S/16] int16: element i at [i%16, i//16], replicated x8."""
    S = idx.shape[0]
    assert S % 16 == 0
    w = np.zeros((16, S // 16), np.int16)
    w[np.arange(S) % 16, np.arange(S) // 16] = idx.astype(np.int16)
    return np.tile(w, (8, 1))


def sentinel_V(att_l):
    a = att_l.reshape(HID)
    s = np.sign(a)
    s[s == 0] = 1.0
    cost = np.where(a > 0, 0.2, 1.0) * np.abs(a)
    m = cost.reshape(HEADS, C).sum(axis=1).min()
    B = float(min(30000.0, 150.0 / max(m, 1e-5)))
    return (-B * s).astype(np.float32)


def _layouts(S_total):
    """Column layouts for the two packed input blobs (keeps per-call arg
    count/bytes small: axon dispatch costs ~0.5ms/arg + ~0.7ms/MB)."""
    bf = {}
    off = 0
    for name, w in [("xT", NLOC_PAD), ("Win", HID), ("Wl", L * 2 * HID),
                    ("Wr", L * 2 * HID), ("attB2", L * 2 * 2 * P),
                    ("idx", S_total // P), ("ew", S_total // P),
                    ("bl", L * HID), ("bin", HID), ("V", L * HID)]:
        bf[name] = (off, w)
        off += w
    f32 = {}
    off2 = 0
    for name, w in [("gin", HID), ("bnin", HID), ("gfn", HID), ("bfn", HID),
                    ("Wg", 2 * HID), ("bg", HID), ("br", L * 2),
                    ("WeT", L * 2), ("outb", L * 2), ("tmask", 1)]:
        f32[name] = (off2, w)
        off2 += w
    return bf, off, f32, off2


# ----------------------------------------------------------------------------
# Bass program
# ----------------------------------------------------------------------------

def _patch_drain_split():
    """Workaround: newer walrus rejects a Drain carrying >1 sync waits
    (CoreV3 CTRL NO_STRUCT allows one). Split the final TileContext drain's
    DMA-sem waits into individual single-wait instructions."""
    import concourse.tile as tile_mod
    if getattr(tile_mod.TileContext, "_drain_split_patched", False):
        return
    import bass_rust
    from concourse.bass import SemaphoreHandle
    from concourse.vector_clock import ScopedClock

    def _drain_and_barrier(self, tick_clock, wait_clock):
        nc = self.nc
        probe = nc.sync.nop(nofuse=True).ins
        wait_clock.add_sem_waits(probe, ScopedClock({None: tick_clock.global_clock}))
        si = probe.sync_info
        waits = list(si.on_wait) if si and si.on_wait else []
        if len(waits) > 1:
            probe.sync_info = bass_rust.SyncInfo(
                on_wait=[waits[0]], on_update=list(si.on_update or []))
            for w in waits[1:]:
                nc.sync.wait_ge(SemaphoreHandle(w.ant_name, w.id), w.wait_value)
        nc.sync.drain()
        nc.all_engine_barrier()
        assert self.sems is not None
        popped = nc._tile_sem_poison_stack.pop()
        assert popped is self._sem_poison
        nc.clear_and_free_semaphores(list(self.sems.allocated().values()))
        nc.all_engine_barrier()

    tile_mod.TileContext._drain_and_barrier = _drain_and_barrier
    tile_mod.TileContext._drain_split_patched = True


def build_program(chunks, S_total, use_bf16=True):
    import os
    STAGE = int(os.environ.get("K_STAGE", "9"))  # debug bisect knob
    GMAX = int(os.environ.get("K_GMAX", "999"))  # max dma_gather instrs
    EWC_ON = os.environ.get("K_EWC", "1") == "1"
    ABS_ON = os.environ.get("K_ABS", "1") == "1"
    gcount = [0]
    import concourse.bacc as bacc
    import concourse.mybir as mybir
    import concourse.tile as tile
    from concourse.masks import make_identity

    f32 = mybir.dt.float32
    bf16 = mybir.dt.bfloat16 if use_bf16 else mybir.dt.float32
    i16 = mybir.dt.int16
    Alu = mybir.AluOpType
    Act = mybir.ActivationFunctionType
    AX = mybir.AxisListType

    _patch_drain_split()
    nc = bacc.Bacc(None, num_devices=NCORES)

    BF, WB, FF, WF = _layouts(S_total)
    blob_bf = nc.declare_dram_parameter("blob_bf", [P, WB], bf16,
                                        isOutput=False)
    blob_f = nc.declare_dram_parameter("blob_f", [P, WF], f32, isOutput=False)
    out_d = nc.declare_dram_parameter("out", [NLOC, HID], f32, isOutput=True)

    def bfs(name):  # bf16 blob slice
        o, w = BF[name]
        return blob_bf[:, o:o + w]

    def ffs(name):
        o, w = FF[name]
        return blob_f[:, o:o + w]

    groups = [list(range(NCORES))]

    with tile.TileContext(nc) as tc, \
            tc.tile_pool(name="const", bufs=1) as cp, \
            tc.tile_pool(name="dram", bufs=1, space="DRAM") as dp, \
            tc.tile_pool(name="work", bufs=2) as sp, \
            tc.tile_pool(name="psum", bufs=4, space="PSUM") as pp, \
            tc.tile_pool(name="psum2", bufs=2, space="PSUM") as pp2:

        # ---- persistent SBUF constants (sliced out of the blobs) ----
        def load(dst_shape, dt, src_ap, tag):
            t = cp.tile(list(dst_shape), dt, tag=tag)
            nc.sync.dma_start(out=t[:], in_=src_ap)
            return t

        Win_sb = load([P, HID], bf16, bfs("Win"), "Win")
        bin_sb = load([1, HID], bf16, bfs("bin")[0:1], "bin")
        gin_sb = load([P, HID], f32, ffs("gin"), "gin")
        bnin_sb = load([P, HID], f32, ffs("bnin"), "bnin")
        gfn_sb = load([P, HID], f32, ffs("gfn"), "gfn")
        bfn_sb = load([P, HID], f32, ffs("bfn"), "bfn")
        Wg_sb = load([P, 2, HID], f32,
                     ffs("Wg").rearrange("p (s h) -> p s h", s=2), "Wg")
        bg_sb = load([1, HID], f32, ffs("bg")[0:1], "bg")
        Wl_sb = load([P, L, 2, HID], bf16,
                     bfs("Wl").rearrange("p (l s h) -> p l s h", l=L, s=2),
                     "Wl")
        Wr_sb = load([P, L, 2, HID], bf16,
                     bfs("Wr").rearrange("p (l s h) -> p l s h", l=L, s=2),
                     "Wr")
        bl_sb = load([1, L, HID], bf16,
                     bfs("bl")[0:1].rearrange("o (l h) -> o l h", l=L), "bl")
        br_sb = load([P, L, 2, 1], f32,
                     ffs("br").rearrange("p (l s) -> p l s ()", l=L), "br")
        # attB2[:, l, s, a, :]: a=0 -> 0.6*att, a=1 -> 0.4*att
        attB_sb = load([P, L, 2, 2, P], bf16,
                       bfs("attB2").rearrange("p (l s a q) -> p l s a q",
                                              l=L, s=2, a=2), "attB")
        WeT_sb = load([P, L, 2, 1], f32,
                      ffs("WeT").rearrange("p (l s) -> p l s ()", l=L), "WeT")
        outb_sb = load([P, L, 2, 1], f32,
                       ffs("outb").rearrange("p (l s) -> p l s ()", l=L),
                       "outb")
        tmask_sb = load([P, 1], f32, ffs("tmask"), "tmask")

        # idx: 16-row wrap packed densely as [128, S/128] in the blob
        # (row j*8+b holds idx16[j, b*Q:(b+1)*Q]); unwrap + replicate x8
        idx_sb = cp.tile([P, S_total // 16], i16, tag="idx")
        o_idx, w_idx = BF["idx"]
        Q = S_total // P
        for k in range(8):
            nc.sync.dma_start(
                out=idx_sb[16 * k:16 * (k + 1), :]
                .rearrange("j (b q) -> j b q", q=Q),
                in_=blob_bf[:, o_idx:o_idx + w_idx].bitcast(i16)
                .rearrange("(j b) q -> j b q", b=8))

        # ew: [128, S/128] in blob -> contiguous row -> 128-replicated DRAM
        # scratch (keeps the big replicated array off the per-call upload)
        ew_scr = dp.tile([P, S_total], bf16, tag="ew_scr")
        o_ew, w_ew = BF["ew"]
        nc.sync.dma_start(
            out=ew_scr[0:1].rearrange("o (p j) -> o p j", p=P),
            in_=blob_bf[:, o_ew:o_ew + w_ew].rearrange("p j -> () p j"))
        reps = 1
        while reps < P:
            nc.sync.dma_start(out=ew_scr[reps:2 * reps, :],
                              in_=ew_scr[0:reps, :])
            reps *= 2

        ident_bf = cp.tile([P, P], bf16, tag="identbf")
        make_identity(nc, ident_bf[:])
        ident_f = cp.tile([P, P], f32, tag="identf")
        make_identity(nc, ident_f[:])
        ones_row_bf = cp.tile([1, P], bf16, tag="onesrbf")
        nc.vector.memset(ones_row_bf[:], 1.0)
        ones_row_f = cp.tile([1, P], f32, tag="onesrf")
        nc.vector.memset(ones_row_f[:], 1.0)
        ones_col_f = cp.tile([P, 1], f32, tag="onescf")
        nc.vector.memset(ones_col_f[:], 1.0)
        one11_f = cp.tile([1, 1], f32, tag="one11")
        nc.vector.memset(one11_f[:], 1.0)
        eps_col = cp.tile([P, 1], f32, tag="epscol")
        nc.vector.memset(eps_col[:], 1e-5)

        WeT_bf = cp.tile([P, L, 2, 1], bf16, tag="WeTbf")
        nc.vector.tensor_copy(out=WeT_bf[:], in_=WeT_sb[:])

        # ---- LN helper (node-major [128, HID] f32 in) ----
        def layer_norm(h_ap, out_ap, gain_ap=None, bias_ap=None):
            mu = sp.tile([P, 1], f32, tag="ln_mu")
            nc.vector.tensor_reduce(out=mu[:], in_=h_ap, axis=AX.X, op=Alu.add)
            nc.vector.tensor_scalar(out=mu[:], in0=mu[:], scalar1=1.0 / HID,
                                    scalar2=None, op0=Alu.mult)
            zc = sp.tile([P, HID], f32, tag="ln_zc")
            nc.vector.tensor_scalar(out=zc[:], in0=h_ap, scalar1=mu[:],
                                    scalar2=None, op0=Alu.subtract)
            sq = sp.tile([P, HID], f32, tag="ln_sq")
            ss = sp.tile([P, 1], f32, tag="ln_ss")
            nc.scalar.activation(out=sq[:], in_=zc[:], func=Act.Square,
                                 accum_out=ss[:])
            nc.vector.tensor_scalar(out=ss[:], in0=ss[:], scalar1=1.0 / HID,
                                    scalar2=None, op0=Alu.mult)
            sd = sp.tile([P, 1], f32, tag="ln_sd")
            nc.scalar.activation(out=sd[:], in_=ss[:], func=Act.Sqrt,
                                 bias=eps_col[:])
            rstd = sp.tile([P, 1], f32, tag="ln_rstd")
            nc.vector.reciprocal(out=rstd[:], in_=sd[:])
            if gain_ap is None:
                nc.vector.tensor_scalar(out=out_ap, in0=zc[:], scalar1=rstd[:],
                                        scalar2=None, op0=Alu.mult)
            else:
                z = sp.tile([P, HID], f32, tag="ln_z")
                nc.vector.tensor_scalar(out=z[:], in0=zc[:], scalar1=rstd[:],
                                        scalar2=None, op0=Alu.mult)
                nc.vector.tensor_tensor(out=z[:], in0=z[:], in1=gain_ap,
                                        op=Alu.mult)
                nc.vector.tensor_tensor(out=out_ap, in0=z[:], in1=bias_ap,
                                        op=Alu.add)

        # ---- input stage: h = LN(gelu(x @ W_in + b_in)) ----
        import contextlib
        scope = nc.named_scope if hasattr(nc, "named_scope") else (
            lambda name: contextlib.nullcontext())
        h_sb = cp.tile([P, NTILE, HID], f32, tag="h")
        o_xt, _ = BF["xT"]
        for i in range(NTILE):
            nrows = min(P, NLOC - i * P)
            xt = sp.tile([P, P], bf16, tag="xt")
            nc.sync.dma_start(out=xt[:],
                              in_=blob_bf[:, o_xt + i * P:o_xt + (i + 1) * P])
            ps_h = pp.tile([P, HID], f32, tag="mm")
            nc.tensor.matmul(out=ps_h[:, :HID],
                             lhsT=xt[:], rhs=Win_sb[:],
                             start=True, stop=False)
            nc.tensor.matmul(out=ps_h[:, :HID], lhsT=ones_row_bf[:],
                             rhs=bin_sb[:], start=False, stop=True)
            hg = sp.tile([P, HID], f32, tag="hg")
            nc.scalar.activation(out=hg[:], in_=ps_h[:, :HID], func=Act.Gelu)
            layer_norm(hg[:], h_sb[:, i], gain_ap=gin_sb[:], bias_ap=bnin_sb[:])
            if nrows < P:
                # zero pad-node rows (partition writes must be 32-aligned,
                # so mask-multiply instead of a partial memset)
                nc.vector.tensor_scalar(out=h_sb[:, i], in0=h_sb[:, i],
                                        scalar1=tmask_sb[:], scalar2=None,
                                        op0=Alu.mult)

        # persistent per-layer tiles
        hnT = cp.tile([P, 2, NLOC_PAD], bf16, tag="hnT")
        xrT = cp.tile([P, 2, NLOC_PAD], bf16, tag="xrT")
        onodeT = cp.tile([P, 2, NLOC_PAD], f32, tag="onodeT")
        nc.vector.memset(onodeT[:], 0.0)
        denT = cp.tile([P, 2, NLOC_PAD], f32, tag="denT")
        table = cp.tile([P, NSTRIPE * HID], bf16, tag="table")
        nc.vector.memset(table[:], 0.0)

        # one gpsimd register per distinct chunk size (to_reg never frees)
        nidx_regs = {}
        for ch in chunks:
            if ch["S"] not in nidx_regs:
                nidx_regs[ch["S"]] = nc.gpsimd.to_reg(ch["S"])

        # ---- layer loop ----
        for l in range(L):
          with scope(f"L{l}_pre"):
            hn = sp.tile([P, NTILE, HID], bf16, tag="hn")
            for i in range(NTILE):
                layer_norm(h_sb[:, i], hn[:, i])
            for i in range(NTILE):
                for s in range(2):
                    ps_t = pp2.tile([P, P], bf16, tag="tr")
                    nc.tensor.transpose(out=ps_t[:],
                                        in_=hn[:, i, s * P:(s + 1) * P],
                                        identity=ident_bf[:])
                    nc.vector.tensor_copy(out=hnT[:, s, i * P:(i + 1) * P],
                                          in_=ps_t[:])

            # xl token rows (node-major) -> DRAM; 2 sentinel rows appended so
            # the AllGather distributes them (keeps ag2 single-writer/Shared)
            xl_dram = dp.tile([NLOC_X, HID], bf16, tag="xl_loc")
            o_v, w_v = BF["V"]
            nc.sync.dma_start(
                out=xl_dram[NLOC:NLOC_X, :],
                in_=blob_bf[0:2, o_v + l * HID:o_v + (l + 1) * HID])
            for i in range(NTILE):
                nrows = min(P, NLOC - i * P)
                ps_xl = pp.tile([P, HID], f32, tag="mm")
                for kc in range(2):
                    nc.tensor.matmul(out=ps_xl[:, :HID],
                                     lhsT=hnT[:, kc, i * P:(i + 1) * P],
                                     rhs=Wl_sb[:, l, kc],
                                     start=(kc == 0), stop=False)
                nc.tensor.matmul(out=ps_xl[:, :HID], lhsT=ones_row_bf[:],
                                 rhs=bl_sb[:, l], start=False, stop=True)
                xl_bf = sp.tile([P, HID], bf16, tag="xl_bf")
                nc.scalar.copy(out=xl_bf[:], in_=ps_xl[:, :HID])
                nc.sync.dma_start(out=xl_dram[i * P:i * P + nrows, :],
                                  in_=xl_bf[:nrows, :])

            # xrT channel-major
            for s in range(2):
                for n0 in range(0, NLOC, 512):
                    nn = min(512, NLOC - n0)
                    ps_xr = pp.tile([P, 512], f32, tag="mm")
                    for kc in range(2):
                        nc.tensor.matmul(
                            out=ps_xr[:, :nn],
                            lhsT=Wr_sb[:, l, kc, s * P:(s + 1) * P],
                            rhs=hnT[:, kc, n0:n0 + nn],
                            start=(kc == 0), stop=(kc == 1))
                    nc.scalar.activation(out=xrT[:, s, n0:n0 + nn],
                                         in_=ps_xr[:, :nn], func=Act.Identity,
                                         bias=br_sb[:, l, s])

          with scope(f"L{l}_ag"):
            # AllGather xl+sentinel rows -> DRAM token store
            # (row == global token id = core*NLOC_X + rank)
            ag2 = dp.tile([NTOK, HID], bf16, tag="ag2", addr_space="Shared")
            nc.gpsimd.collective_compute(
                "AllGather", Alu.bypass, replica_groups=groups,
                ins=[xl_dram[:]], outs=[ag2[:]])
          with scope(f"L{l}_tbl"):
            # token-interleaved stripes: token t -> partition t%128, stripe
            # t//128 (SBUF-source gather beats HBM-source: 512B random HBM
            # reads measured ~40% slower end-to-end)
            nc.sync.dma_start(
                out=table[:, 0:78 * HID].rearrange("p (r c) -> p r c", c=HID),
                in_=ag2[0:78 * P, :].rearrange("(r p) c -> p r c", p=P))
            nc.sync.dma_start(out=table[0:32, 78 * HID:79 * HID],
                              in_=ag2[78 * P:78 * P + 32, :])

            # ---- edge chunks (super-chunks of SUPER 512-slot chunks) ----
          with scope(f"L{l}_edges"):
            CT = CHUNK_TARGET
            for sc0 in range(0, len(chunks), SUPER):
                if STAGE < 1:
                    break
                sch = chunks[sc0:sc0 + SUPER]
                G = len(sch)
                soff = sc0 * CT
                xlg = sp.tile([P, G, 2, CT], bf16, tag="xlg")
                for gi, ch in enumerate(sch):
                    if gcount[0] < GMAX:
                        gcount[0] += 1
                        nc.gpsimd.dma_gather(
                            out_ap=xlg[:, gi], in_ap=table[:],
                            idxs_ap=idx_sb[:, (soff + gi * CT) // 16:
                                           (soff + (gi + 1) * CT) // 16],
                            num_idxs=CT, num_idxs_reg=nidx_regs[CT],
                            elem_size=HID, transpose=True,
                            sbuf_tokens_per_rank=P,
                            sbuf_free_dim_per_rank=HID * 2,
                        )
                    else:
                        nc.vector.memset(xlg[:, gi], 0.0)
                ewc = sp.tile([P, G, CT], bf16, tag="ewc")
                if EWC_ON:
                    nc.sync.dma_start(
                        out=ewc[:],
                        in_=ew_scr[:, soff:soff + G * CT]
                        .rearrange("p (g x) -> p g x", x=CT))
                else:
                    nc.vector.memset(ewc[:], 0.0)
                if STAGE < 2:
                    continue
                u = sp.tile([P, G, 2, CT], bf16, tag="u")
                for s in range(2):
                    nc.vector.scalar_tensor_tensor(
                        out=u[:, :, s], in0=ewc[:], scalar=WeT_bf[:, l, s],
                        in1=xlg[:, :, s], op0=Alu.mult, op1=Alu.add)
                for gi, ch in enumerate(sch):
                    for s in range(2):
                        for r in ch["runs"]:
                            n0 = ch["node0"] + r["node_off"]
                            nn = r["n"]
                            pad = r["pad"]
                            uv = u[:, gi, s,
                                   r["slot_off"]:r["slot_off"] + nn * pad] \
                                .rearrange("p (n k) -> p n k", k=pad)
                            nc.vector.tensor_tensor(
                                out=uv, in0=uv,
                                in1=xrT[:, s, n0:n0 + nn]
                                .to_broadcast([P, nn, pad]),
                                op=Alu.add)
                if STAGE < 3:
                    continue
                a_t = sp.tile([P, G, 2, CT], bf16, tag="a")
                if ABS_ON:
                    # logits = 0.6*att@u + 0.4*att@|u|  (== att@lrelu(u, 0.2));
                    # |u| = clear the sign bit, int16 tensor_scalar runs at 4x
                    # (ACT Abs regressed: extra cross-engine hop on the u ->
                    # logits critical path)
                    au = sp.tile([P, G, 2, CT], bf16, tag="lr")
                    nc.vector.tensor_scalar(
                        out=au[:].bitcast(i16), in0=u[:].bitcast(i16),
                        scalar1=0x7FFF, scalar2=None, op0=Alu.bitwise_and)
                    for gi in range(G):
                        for s in range(2):
                            ps_a = pp.tile([P, CT], f32, tag="mm")
                            nc.tensor.matmul(out=ps_a[:],
                                             lhsT=attB_sb[:, l, s, 0],
                                             rhs=u[:, gi, s],
                                             start=True, stop=False)
                            nc.tensor.matmul(out=ps_a[:],
                                             lhsT=attB_sb[:, l, s, 1],
                                             rhs=au[:, gi, s],
                                             start=False, stop=True)
                            nc.scalar.activation(out=a_t[:, gi, s],
                                                 in_=ps_a[:], func=Act.Exp)
                else:
                    # leaky relu on DVE, then att = (0.6att + 0.4att) in two
                    # accumulating matmuls
                    lr = sp.tile([P, G, 2, CT], bf16, tag="lr")
                    nc.vector.scalar_tensor_tensor(
                        out=lr[:], in0=u[:], scalar=0.2, in1=u[:],
                        op0=Alu.mult, op1=Alu.max)
                    for gi in range(G):
                        for s in range(2):
                            ps_a = pp.tile([P, CT], f32, tag="mm")
                            nc.tensor.matmul(out=ps_a[:],
                                             lhsT=attB_sb[:, l, s, 0],
                                             rhs=lr[:, gi, s],
                                             start=True, stop=False)
                            nc.tensor.matmul(out=ps_a[:],
                                             lhsT=attB_sb[:, l, s, 1],
                                             rhs=lr[:, gi, s],
                                             start=False, stop=True)
                            nc.scalar.activation(out=a_t[:, gi, s],
                                                 in_=ps_a[:], func=Act.Exp)
                if STAGE < 4:
                    continue
                m_t = sp.tile([P, G, 2, CT], bf16, tag="u")
                nc.vector.tensor_tensor(out=m_t[:], in0=a_t[:], in1=xlg[:],
                                        op=Alu.mult)
                for gi, ch in enumerate(sch):
                    for s in range(2):
                        for r in ch["runs"]:
                            no = r["node_off"]
                            nn = r["n"]
                            pad = r["pad"]
                            n0 = ch["node0"] + no
                            av = a_t[:, gi, s,
                                     r["slot_off"]:r["slot_off"] + nn * pad] \
                                .rearrange("p (n k) -> p n k", k=pad)
                            nc.vector.tensor_reduce(out=denT[:, s, n0:n0 + nn],
                                                    in_=av, axis=AX.X,
                                                    op=Alu.add)
                            mv = m_t[:, gi, s,
                                     r["slot_off"]:r["slot_off"] + nn * pad] \
                                .rearrange("p (n k) -> p n k", k=pad)
                            nc.vector.tensor_reduce(
                                out=onodeT[:, s, n0:n0 + nn],
                                in_=mv, axis=AX.X, op=Alu.add)

            # alpha normalization (deferred), out_b bias, h += transpose(onodeT)
          with scope(f"L{l}_apply"):
            for s in range(2):
                nc.vector.reciprocal(out=denT[:, s, :NLOC],
                                     in_=denT[:, s, :NLOC])
                nc.vector.tensor_tensor(out=onodeT[:, s, :NLOC],
                                        in0=onodeT[:, s, :NLOC],
                                        in1=denT[:, s, :NLOC], op=Alu.mult)
                nc.vector.tensor_scalar(out=onodeT[:, s, :NLOC],
                                        in0=onodeT[:, s, :NLOC],
                                        scalar1=outb_sb[:, l, s], scalar2=None,
                                        op0=Alu.add)
                # transpose 4 node-tiles into one PSUM tile, add in one DVE op
                for i0 in range(0, NTILE, 4):
                    gt = min(4, NTILE - i0)
                    ps_t4 = pp.tile([P, 4 * P], f32, tag="mm")
                    for g in range(gt):
                        nc.tensor.transpose(
                            out=ps_t4[:, g * P:(g + 1) * P],
                            in_=onodeT[:, s, (i0 + g) * P:(i0 + g + 1) * P],
                            identity=ident_f[:])
                    nc.vector.tensor_tensor(
                        out=h_sb[:, i0:i0 + gt, s * P:(s + 1) * P],
                        in0=h_sb[:, i0:i0 + gt, s * P:(s + 1) * P],
                        in1=ps_t4[:, 0:gt * P]
                        .rearrange("p (g q) -> p g q", q=P),
                        op=Alu.add)

        # ---- final: context gate + LN ----
        fin_ctx = scope("fin")
        fin_ctx.__enter__()
        ps_ctx = pp2.tile([1, HID], f32, tag="sm")
        for i in range(NTILE):
            nc.tensor.matmul(out=ps_ctx[:], lhsT=ones_col_f[:], rhs=h_sb[:, i],
                             start=(i == 0), stop=(i == NTILE - 1))
        ctx_sb = sp.tile([1, HID], f32, tag="ctx")
        nc.vector.tensor_copy(out=ctx_sb[:], in_=ps_ctx[:])
        c_in = dp.tile([1, HID], f32, tag="c_in")
        c_out = dp.tile([1, HID], f32, tag="c_out", addr_space="Shared")
        nc.sync.dma_start(out=c_in[:], in_=ctx_sb[:])
        nc.gpsimd.collective_compute("AllReduce", Alu.add, replica_groups=groups,
                                     ins=[c_in[:]], outs=[c_out[:]])
        nc.sync.dma_start(out=ctx_sb[:], in_=c_out[:])
        nc.scalar.mul(out=ctx_sb[:], in_=ctx_sb[:], mul=1.0 / N)
        ctxT = sp.tile([P, 2, 1], f32, tag="ctxT")
        for s in range(2):
            ps_ct = pp2.tile([P, 1], f32, tag="sm")
            nc.tensor.matmul(out=ps_ct[:], lhsT=ctx_sb[:, s * P:(s + 1) * P],
                             rhs=one11_f[:], start=True, stop=True)
            nc.vector.tensor_copy(out=ctxT[:, s], in_=ps_ct[:])
        ps_g = pp2.tile([1, HID], f32, tag="sm")
        for s in range(2):
            nc.tensor.matmul(out=ps_g[:], lhsT=ctxT[:, s], rhs=Wg_sb[:, s],
                             start=(s == 0), stop=False)
        nc.tensor.matmul(out=ps_g[:], lhsT=one11_f[:], rhs=bg_sb[:],
                         start=False, stop=True)
        gate = sp.tile([1, HID], f32, tag="gate")
        nc.scalar.activation(out=gate[:], in_=ps_g[:], func=Act.Sigmoid)
        gc = sp.tile([1, HID], f32, tag="gc")
        nc.vector.tensor_tensor(out=gc[:], in0=gate[:], in1=ctx_sb[:],
                                op=Alu.mult)
        ps_gc = pp.tile([P, HID], f32, tag="mm")
        nc.tensor.matmul(out=ps_gc[:, :HID], lhsT=ones_row_f[:], rhs=gc[:],
                         start=True, stop=True)
        gc_sb = sp.tile([P, HID], f32, tag="gc_sb")
        nc.vector.tensor_copy(out=gc_sb[:], in_=ps_gc[:, :HID])
        hf = cp.tile([P, HID], f32, tag="hf")
        for i in range(NTILE):
            nrows = min(P, NLOC - i * P)
            nc.vector.tensor_tensor(out=h_sb[:, i], in0=h_sb[:, i],
                                    in1=gc_sb[:], op=Alu.add)
            layer_norm(h_sb[:, i], hf[:], gain_ap=gfn_sb[:], bias_ap=bfn_sb[:])
            nc.sync.dma_start(out=out_d[i * P:i * P + nrows, :],
                              in_=hf[:nrows, :])
        fin_ctx.__exit__(None, None, None)

    nc.finalize()
    return nc


# ----------------------------------------------------------------------------
# host wrapper
# ----------------------------------------------------------------------------

_CACHE = {}


def make_in_maps(inputs, sched, use_bf16=True):
    import ml_dtypes
    bfnp = ml_dtypes.bfloat16 if use_bf16 else np.float32

    x = np.asarray(inputs["x"], np.float32)
    g_ly = np.asarray(inputs["g_ly"], np.float32)
    bn_ly = np.asarray(inputs["bn_ly"], np.float32)
    Wl = np.asarray(inputs["Wl"], np.float32)
    bl = np.asarray(inputs["bl"], np.float32)
    Wr = np.asarray(inputs["Wr"], np.float32)
    br = np.asarray(inputs["br"], np.float32)
    We = np.asarray(inputs["We"], np.float32)
    att = np.asarray(inputs["att"], np.float32)
    out_b = np.asarray(inputs["out_b"], np.float32)

    S_total = sched["S_total"]
    BF, WB, FF, WF = _layouts(S_total)

    Wl_eff = g_ly[:, :, None] * Wl
    Wr_eff = g_ly[:, :, None] * Wr
    bl_eff = np.einsum("lk,lkc->lc", bn_ly, Wl) + bl
    br_eff = np.einsum("lk,lkc->lc", bn_ly, Wr) + br
    # [L, 2, 128, HID] -> [128, L, 2, HID]
    Wl_k = Wl_eff.reshape(L, 2, 128, HID).transpose(2, 0, 1, 3)
    Wr_k = Wr_eff.reshape(L, 2, 128, HID).transpose(2, 0, 1, 3)

    # attB[c_l, l, s, p] = att[8s + p//16, c_l%16] if c_l//16 == p//16
    attB = np.zeros((128, L, 2, 128), np.float32)
    cl = np.arange(128)
    for l in range(L):
        for s in range(2):
            for p in range(128):
                sel = (cl // 16) == (p // 16)
                attB[sel, l, s, p] = att[l, 8 * s + p // 16, cl[sel] % 16]
    # [128, L, 2, a, 128]: a=0 -> 0.6*att, a=1 -> 0.4*att
    attB2 = np.stack([0.6 * attB, 0.4 * attB], axis=3)
    V = np.stack([sentinel_V(att[l]) for l in range(L)])  # [L, HID]

    def put(blob, layout, name, arr, nparts=P):
        o, w = layout[name]
        a = np.asarray(arr).reshape(nparts, -1)
        assert a.shape[1] == w, (name, a.shape, w)
        blob[:nparts, o:o + w] = a

    bbf = np.zeros((P, WB), bfnp)
    put(bbf, BF, "Win", np.asarray(inputs["W_in"], np.float32).astype(bfnp))
    put(bbf, BF, "Wl", Wl_k.astype(bfnp))
    put(bbf, BF, "Wr", Wr_k.astype(bfnp))
    put(bbf, BF, "attB2", attB2.astype(bfnp))
    put(bbf, BF, "bl", bl_eff.reshape(1, -1).astype(bfnp), nparts=1)
    put(bbf, BF, "bin",
        np.asarray(inputs["b_in"], np.float32).reshape(1, -1).astype(bfnp),
        nparts=1)
    put(bbf, BF, "V", np.tile(V.reshape(1, -1), (16, 1)).astype(bfnp),
        nparts=16)

    bf32 = np.zeros((P, WF), np.float32)
    for nm, key in [("gin", "g_in"), ("bnin", "bn_in"), ("gfn", "g_fn"),
                    ("bfn", "b_fn")]:
        put(bf32, FF, nm,
            np.tile(np.asarray(inputs[key], np.float32)[None], (P, 1)))
    put(bf32, FF, "Wg", np.asarray(inputs["Wg"], np.float32)
        .reshape(2, 128, HID).transpose(1, 0, 2).reshape(P, -1))
    put(bf32, FF, "bg",
        np.tile(np.asarray(inputs["bg"], np.float32).reshape(1, -1), (P, 1)))
    put(bf32, FF, "br", br_eff.reshape(L, 2, 128).transpose(2, 0, 1)
        .reshape(P, -1).astype(np.float32))
    put(bf32, FF, "WeT", We.reshape(L, 2, 128).transpose(2, 0, 1)
        .reshape(P, -1).astype(np.float32))
    put(bf32, FF, "outb", out_b.reshape(L, 2, 128).transpose(2, 0, 1)
        .reshape(P, -1).astype(np.float32))
    tm = np.ones((128, 1), np.float32)
    tm[NLOC - (NTILE - 1) * 128:] = 0.0
    put(bf32, FF, "tmask", tm)

    in_maps = []
    for c in range(NCORES):
        b = bbf.copy()
        xc = x[c * NLOC + sched["perms"][c]]          # [NLOC, IN]
        xT = np.zeros((P, NLOC_PAD), np.float32)
        xT[:IN, :NLOC] = xc.T
        put(b, BF, "xT", xT.astype(bfnp))
        idx16 = wrap_idx16(sched["src_tok"][c])[:16].view(np.uint16)
        put(b, BF, "idx",
            idx16.reshape(16, 8, S_total // P).reshape(P, S_total // P)
            .view(bfnp))
        put(b, BF, "ew",
            sched["ew_slot"][c].astype(bfnp).reshape(P, -1))
        in_maps.append({"blob_bf": b, "blob_f": bf32})
    return in_maps


def _get_program(inputs, use_bf16=True):
    key = ("prog", use_bf16)
    if key not in _CACHE:
        sched = build_schedule(np.asarray(inputs["edge_index"]),
                               np.asarray(inputs["edge_weight"]))
        nc = build_program(sched["chunks"], sched["S_total"],
                           use_bf16=use_bf16)
        _CACHE[key] = (nc, sched)
    return _CACHE[key]


def kernel(**inputs):
    from concourse.bass_utils import run_bass_kernel_spmd

    nc, sched = _get_program(inputs)
    in_maps = make_in_maps(inputs, sched)
    res = run_bass_kernel_spmd(nc, in_maps, list(range(NCORES))).results
    out = np.zeros((N, HID), np.float32)
    for c in range(NCORES):
        out[c * NLOC + sched["perms"][c]] = res[c]["out"]
    return out

